# revision 48
# baseline (speedup 1.0000x reference)
"""Single-head causal self-attention on 8 Trainium2 NeuronCores.

Problem: x[8, 2048, 1024], Wq/Wk/Wv[1024, 64] ->
  out[b] = softmax(causal((x[b]@Wq) @ (x[b]@Wk)^T / 8)) @ (x[b]@Wv)

Sharding: data-parallel over batch B=8, one batch element per core; weights
replicated. All matmul operands are bf16 (1 PE cycle/row vs 4 for fp32, and
half the DMA bytes); accumulation stays fp32 in PSUM.

Per-core scheme:
  - host pre-packs x[b]^T as [128, 8, 2048] bf16 so every DMA line is long
    and contiguous per partition; input DMAs are issued upfront on SP in the
    order compute consumes them (wqk, x0, wv, x1, x2, x3)
  - [k^T;q^T] = Wkq^T @ x^T  (W-stationary, PSUM [128,512] per t-chunk,
    evacuated by one full-width DVE copy; the k half additionally needs a
    partition shift, done by a scalar-engine copy for chunks 0-2 and an
    SBUF->SBUF DMA on the idle Pool queue for the last chunk); V = x @ Wv
    in natural [t, 64] layout (x-stationary: 64-col outputs, half the PE
    cycles of the W-stationary form), in its own PSUM bank
  - S^T[j-tile, q-chunk] = (k^T tile)^T @ q^T, causal blocks only;
    off-diagonal j-tiles are computed in PAIRS into a 2-bank PSUM tile so a
    single ACT exp instruction covers 1024 columns (halves ACT's fixed
    per-instruction access overhead); diagonal tiles stay single, sliced at
    the causal boundary, and are masked with a bf16 triangle on DVE
  - out[q-tile, 65] += P^T-block^T @ V[j]  (AV in natural layout: 65 output
    cols per block; col 64 of V is ones, making the softmax denominator a
    free by-product); rows normalized with DVE reciprocal (per-tile for the
    last chunk to shorten the drain)
  - attention units from ALL chunks form one software-pipelined stream; the
    next chunk's projections are emitted between units so neither PE nor ACT
    drains at chunk boundaries (engines execute strictly in emission order)
  - warmup matmuls on junk data ramp the PE p-state to full clock while the
    first x chunk is in flight; the Exp table is preloaded at t~0
"""

import numpy as np
import ml_dtypes

import concourse.bass as bass
import concourse.mybir as mybir
import concourse.tile as tile
from concourse import bacc
from concourse.bass_utils import run_bass_kernel_spmd
from concourse.masks import make_upper_triangular

N_CORES = 8
B, T, C, D = 8, 2048, 1024, 64
CT = C // 128           # 8 contraction tiles
NT = T // 128           # 16 row tiles
QCHUNK = 512
NQC = T // QCHUNK       # 4 q-chunks
JPER = QCHUNK // 128    # 4 j-tiles per q-chunk
SCALE = float(1.0 / np.sqrt(D))
N_WARMUP = 34           # 128-col PE p-state ramp matmuls during initial DMA

FP = mybir.dt.float32
BF = mybir.dt.bfloat16


def build_nc():
    nc = bacc.Bacc("TRN2", target_bir_lowering=False)
    xT_h = nc.dram_tensor("xT", [128, CT, T], BF, kind="ExternalInput")
    wqk_h = nc.dram_tensor("wqk", [128, CT, 128], BF, kind="ExternalInput")
    wv_h = nc.dram_tensor("wv", [128, CT, D], BF, kind="ExternalInput")
    y_h = nc.dram_tensor("y", [128, NT, D], FP, kind="ExternalOutput")

    with tile.TileContext(nc) as tc:
        with (
            tc.tile_pool(name="const", bufs=1) as const,
            tc.tile_pool(name="pt", bufs=6) as ptp,      # [128,2,512] bf16
            tc.tile_pool(name="dve", bufs=2) as dvp,
            tc.tile_pool(name="ps_s2", bufs=2, space="PSUM") as ps_s2,  # 2x2 banks
            tc.tile_pool(name="ps_pq", bufs=2, space="PSUM") as ps_pq,  # 2 banks
            tc.tile_pool(name="ps_pv", bufs=1, space="PSUM") as ps_pv,  # 1 bank
            tc.tile_pool(name="ps_o", bufs=1, space="PSUM") as ps_o,    # 1 bank
        ):
            # ---- constants (wu first: it gates the PE warmup) ----
            wu = const.tile([128, 128], BF, tag="wu")
            nc.gpsimd.memset(wu, 0.0)
            tri = const.tile([128, 128], BF, tag="tri")  # tri[p,f]=1.0 iff f>=p
            make_upper_triangular(nc, tri, val=1.0, diag=True)
            # preload the Exp activation table while DMAs are in flight
            dum = const.tile([1, 2], BF, tag="dum")
            nc.scalar.activation(
                dum[0:1, 0:1], wu[0:1, 0:1], mybir.ActivationFunctionType.Exp
            )

            wqk_sb = const.tile([128, CT, 128], BF, tag="wqk")
            wv_sb = const.tile([128, CT, D], BF, tag="wv")
            xT_sb = const.tile([128, CT, T], BF, tag="xT")

            # input DMAs, all upfront on SP, in consumption order; ct-quarters
            # let the qk contraction start on the first arriving piece
            def xdma(cu):
                sl = slice(cu * QCHUNK, (cu + 1) * QCHUNK)
                for q in range(4):
                    nc.sync.dma_start(
                        out=xT_sb[:, 2 * q : 2 * q + 2, sl],
                        in_=xT_h[:, 2 * q : 2 * q + 2, sl],
                    )

            nc.sync.dma_start(out=wqk_sb, in_=wqk_h[:, :, :])
            xdma(0)
            nc.sync.dma_start(out=wv_sb, in_=wv_h[:, :, :])
            for cu in range(1, NQC):
                xdma(cu)

            # q/k live on partitions 64:128 (psum high half evacuates with no
            # partition shift); k's low half goes through a staging tile and an
            # SBUF->SBUF DMA (the only engine-free way to shift partitions)
            qT = const.tile([128, T], BF, tag="qT")
            kT = const.tile([128, T], BF, tag="kT")
            kst = const.tile([64, T], BF, tag="kst")
            V = const.tile([128, NT, D + 1], BF, tag="V")  # col D = ones
            nc.gpsimd.memset(V[:, :, D], 1.0)
            out_sb = const.tile([128, NT, D], FP, tag="out")

            # ---- PE p-state warmup while the first DMA is in flight ----
            ps_junk = ps_s2.tile([128, 2, QCHUNK], FP, tag="s2")
            for w in range(N_WARMUP):
                nc.tensor.matmul(
                    ps_junk[:, 0, 0:128], wu, wu, start=True, stop=True
                )

            proj_psum = {}

            proj_psums = {}

            def proj_qk(cu, lo_ct, hi_ct):
                """qk (W-stationary), emitted in ct-halves so the PE order can
                match the x-quarter DMA arrivals; on the last half, q/k are
                evacuated to bf16: one full-width DVE copy (q rows 64: feed S
                directly), k rows :64 partition-shifted into kT by an
                SBUF->SBUF DMA on the idle Pool queue."""
                sl = slice(cu * QCHUNK, (cu + 1) * QCHUNK)
                if lo_ct == 0:
                    proj_psums[cu] = ps_pq.tile(
                        [128, QCHUNK], FP, tag="pq", name=f"p_qk{cu}"
                    )
                p_qk = proj_psums[cu]
                for ct in range(lo_ct, hi_ct):
                    nc.tensor.matmul(
                        p_qk,
                        wqk_sb[:, ct, :],
                        xT_sb[:, ct, sl],
                        start=(ct == 0),
                        stop=(ct == CT - 1),
                    )
                if hi_ct < CT:
                    return
                nc.vector.tensor_copy(qT[:, sl], p_qk)
                if cu < NQC - 1:
                    # ACT has boundary idle here, and the direct shifted copy
                    # is the lowest-latency path to kT
                    nc.scalar.copy(kT[64:128, sl], p_qk[0:64, :])
                else:
                    # last chunk: ACT is saturated, but the DMA engines are
                    # free (all x transfers done) - shift via SBUF->SBUF DMA
                    nc.gpsimd.dma_start(out=kT[64:128, sl], in_=qT[0:64, sl])

            def proj_v(cu):
                """v (x-stationary, natural layout) + evacuation into V."""
                p_v = ps_pv.tile([128, JPER, D], FP, tag="pv")
                for tt in range(JPER):
                    tsl = slice(cu * QCHUNK + tt * 128, cu * QCHUNK + (tt + 1) * 128)
                    for ct in range(CT):
                        nc.tensor.matmul(
                            p_v[:, tt, :],
                            xT_sb[:, ct, tsl],
                            wv_sb[:, ct, :],
                            start=(tt == 0 and ct == 0),
                            stop=(tt == JPER - 1 and ct == CT - 1),
                            skip_group_check=True,
                        )
                nc.vector.tensor_copy(V[:, cu * JPER : (cu + 1) * JPER, 0:D], p_v)

            # ---- global attention unit stream: all units are j-tile PAIRS --
            # unit key (cu, j0): j-tiles (j0, j0+1) against q-chunk cu
            units = {}
            for cu in range(NQC):
                for j in range(0, (cu + 1) * JPER, 2):
                    units[(cu, j)] = (cu, (j, j + 1))

            p_outs = {}

            def s_unit(key):
                cu, u = units[key]
                # both tiles of the pair are computed from the pair's lowest
                # causal column so a single exp AP covers them; columns left
                # of a tile's own boundary are never read by AV
                lo = max(u[0] - cu * JPER, 0) * 128
                p_s = ps_s2.tile([128, 2, QCHUNK], FP, tag="s2")
                pt = ptp.tile([128, 2, QCHUNK], BF, tag="pt")
                for z, j in enumerate(u):
                    nc.tensor.matmul(
                        p_s[:, z, lo:QCHUNK],
                        kT[64:128, j * 128 : (j + 1) * 128],
                        qT[64:128, cu * QCHUNK + lo : (cu + 1) * QCHUNK],
                        start=True,
                        stop=True,
                    )
                nc.scalar.activation(
                    pt[:, :, lo:QCHUNK],
                    p_s[:, :, lo:QCHUNK],
                    mybir.ActivationFunctionType.Exp,
                    scale=SCALE,
                )
                for z, j in enumerate(u):
                    i_d = j - cu * JPER
                    if i_d >= 0:
                        nc.vector.tensor_mul(
                            pt[:, z, i_d * 128 : (i_d + 1) * 128],
                            pt[:, z, i_d * 128 : (i_d + 1) * 128],
                            tri,
                        )
                return pt

            def av_unit(key, pt):
                cu, u = units[key]
                n_jt = cu * JPER + JPER
                if key[1] == 0:
                    p_outs[cu] = ps_o.tile([128, JPER, D + 1], FP, tag="o", name=f"p_out{cu}")
                p_out = p_outs[cu]
                for z, j in enumerate(u):
                    pj = pt[:, z, :]
                    i_d = j - cu * JPER
                    for qi in range(max(i_d, 0), JPER):
                        nc.tensor.matmul(
                            p_out[:, qi, :],
                            pj[:, qi * 128 : (qi + 1) * 128],
                            V[:, j, :],
                            start=(j == 0 and qi == 0),
                            stop=(j == n_jt - 1 and qi == JPER - 1),
                            skip_group_check=True,
                        )

            def finalize(cu, lo, hi):
                p_out = p_outs[cu]
                rec = dvp.tile([128, hi - lo], FP, tag=f"rec{hi - lo}")
                nc.vector.reciprocal(rec, p_out[:, lo:hi, D])
                for qi in range(lo, hi):
                    nc.vector.tensor_scalar_mul(
                        out_sb[:, cu * JPER + qi, :],
                        p_out[:, qi, 0:D],
                        rec[:, qi - lo : qi - lo + 1],
                    )
                nc.sync.dma_start(
                    out=y_h[:, cu * JPER + lo : cu * JPER + hi, :],
                    in_=out_sb[:, cu * JPER + lo : cu * JPER + hi, :],
                )

            # ---- explicit hand-scheduled action stream -------------------
            # Engines execute in emission order, so cross-chunk interleaving
            # is encoded directly: diagonal units (which wait on the staged
            # kT DMA) are deferred behind later chunks' off-diagonal units;
            # qk halves are placed where their x quarters have landed.
            S, AV = "s", "av"
            actions = [
                ("qk", 0, 0, 8), ("v", 0),
                ("qk", 1, 0, 2), (S, 0, 0), ("qk", 1, 2, 4), (S, 0, 2),
                ("qk", 1, 4, 6), ("qk", 1, 6, 8),
                (AV, 0, 0), (AV, 0, 2), ("fin", 0, 0, 4),
                ("qk", 2, 0, 2), ("v", 1),
                (S, 1, 0), (S, 1, 2),
                ("qk", 2, 2, 4), ("qk", 2, 4, 6),
                (AV, 1, 0), (AV, 1, 2),
                ("qk", 2, 6, 8),
                (S, 2, 0),
                ("qk", 3, 0, 2), ("v", 2),
                (S, 1, 4), (S, 2, 2),
                ("qk", 3, 2, 4),
                (AV, 1, 4),
                ("qk", 3, 4, 6),
                (S, 1, 6), (S, 2, 4),
                ("qk", 3, 6, 8),
                (AV, 1, 6), ("fin", 1, 0, 4),
                (S, 3, 0), (S, 2, 6),
                (AV, 2, 0), (AV, 2, 2), (AV, 2, 4),
                (S, 3, 2), (S, 2, 8),
                ("v", 3),
                (AV, 2, 6),
                (S, 3, 4), (S, 2, 10),
                (AV, 2, 8), (AV, 2, 10), ("fin", 2, 0, 4),
                (S, 3, 6), (S, 3, 8),
                (AV, 3, 0), (AV, 3, 2),
                (S, 3, 10), (S, 3, 12),
                (AV, 3, 4), (AV, 3, 6), (AV, 3, 8),
                (S, 3, 14),
                (AV, 3, 10), (AV, 3, 12), ("fin", 3, 0, 2),
                (AV, 3, 14), ("fin", 3, 2, 4),
            ]
            pts = {}
            for act in actions:
                if act[0] == "qk":
                    proj_qk(act[1], act[2], act[3])
                elif act[0] == "v":
                    proj_v(act[1])
                elif act[0] == S:
                    pts[act[1:]] = s_unit(act[1:])
                elif act[0] == AV:
                    av_unit(act[1:], pts.pop(act[1:]))
                else:
                    finalize(act[1], act[2], act[3])
            assert not pts

    nc.finalize()
    return nc


_NC_CACHE = None
LAST_RESULTS = None


def _pack(w, cols):
    # [C, cols] -> [128, CT, cols] with partition p holding rows {ct*128+p}
    return np.ascontiguousarray(
        np.asarray(w, np.float32).reshape(CT, 128, cols).transpose(1, 0, 2)
    ).astype(ml_dtypes.bfloat16)


def kernel(x, Wq, Wk, Wv, trace=False, **run_kwargs):
    global _NC_CACHE, LAST_RESULTS
    x = np.asarray(x, dtype=np.float32)
    # k in the psum low half, q in the high half (see build_nc)
    wqk = _pack(np.concatenate([np.asarray(Wk, np.float32),
                                np.asarray(Wq, np.float32)], axis=1), 128)
    wv = _pack(Wv, D)

    if _NC_CACHE is None:
        _NC_CACHE = build_nc()
    nc = _NC_CACHE

    in_maps = []
    for b in range(N_CORES):
        # xT[p, ct, t] = x[b, t, ct*128+p]
        xT = np.ascontiguousarray(
            x[b].T.reshape(CT, 128, T).transpose(1, 0, 2)
        ).astype(ml_dtypes.bfloat16)
        in_maps.append({"xT": xT, "wqk": wqk, "wv": wv})

    res = run_bass_kernel_spmd(
        nc, in_maps, core_ids=list(range(N_CORES)), trace=trace, **run_kwargs
    )
    LAST_RESULTS = res
    out = np.empty((N_CORES, T, D), dtype=np.float32)
    for b in range(N_CORES):
        y = res.results[b]["y"]  # [128, NT, D]
        out[b] = np.asarray(y, dtype=np.float32).transpose(1, 0, 2).reshape(T, D)
    return out


if __name__ == "__main__":
    rng = np.random.default_rng(0)
    x = rng.standard_normal((B, T, C), dtype=np.float32)
    s = 1.0 / np.sqrt(C)
    Wq = rng.standard_normal((C, D), dtype=np.float32) * s
    Wk = rng.standard_normal((C, D), dtype=np.float32) * s
    Wv = rng.standard_normal((C, D), dtype=np.float32) * s
    out = kernel(x, Wq, Wk, Wv)
    print("out", out.shape, out.dtype, float(np.abs(out).max()))


# revision 64
# speedup vs baseline: 1.0260x; 1.0260x over previous
"""Single-head causal self-attention on 8 Trainium2 NeuronCores.

Problem: x[8, 2048, 1024], Wq/Wk/Wv[1024, 64] ->
  out[b] = softmax(causal((x[b]@Wq) @ (x[b]@Wk)^T / 8)) @ (x[b]@Wv)

Sharding: data-parallel over batch B=8, one batch element per core; weights
replicated. All matmul operands are bf16 (1 PE cycle/row vs 4 for fp32, and
half the DMA bytes); accumulation stays fp32 in PSUM.

Per-core scheme:
  - host pre-packs x[b]^T as [128, 8, 2048] bf16 so every DMA line is long
    and contiguous per partition; input DMAs are issued upfront on SP in the
    order compute consumes them (wqk, x0, wv, x1, x2, x3)
  - [k^T;q^T] = Wkq^T @ x^T  (W-stationary, PSUM [128,512] per t-chunk,
    evacuated by one full-width DVE copy; the k half additionally needs a
    partition shift, done by a scalar-engine copy for chunks 0-2 and an
    SBUF->SBUF DMA on the idle Pool queue for the last chunk); V = x @ Wv
    in natural [t, 64] layout (x-stationary: 64-col outputs, half the PE
    cycles of the W-stationary form), in its own PSUM bank
  - S^T[j-tile, q-chunk] = (k^T tile)^T @ q^T, causal blocks only;
    off-diagonal j-tiles are computed in PAIRS into a 2-bank PSUM tile so a
    single ACT exp instruction covers 1024 columns (halves ACT's fixed
    per-instruction access overhead); diagonal tiles stay single, sliced at
    the causal boundary, and are masked with a bf16 triangle on DVE
  - out[q-tile, 65] += P^T-block^T @ V[j]  (AV in natural layout: 65 output
    cols per block; col 64 of V is ones, making the softmax denominator a
    free by-product); rows normalized with DVE reciprocal (per-tile for the
    last chunk to shorten the drain)
  - attention units from ALL chunks form one software-pipelined stream; the
    next chunk's projections are emitted between units so neither PE nor ACT
    drains at chunk boundaries (engines execute strictly in emission order)
  - warmup matmuls on junk data ramp the PE p-state to full clock while the
    first x chunk is in flight; the Exp table is preloaded at t~0
"""

import numpy as np
import ml_dtypes

import concourse.bass as bass
import concourse.mybir as mybir
import concourse.tile as tile
from concourse import bacc
from concourse.bass_utils import run_bass_kernel_spmd
from concourse.masks import make_upper_triangular

N_CORES = 8
B, T, C, D = 8, 2048, 1024, 64
CT = C // 128           # 8 contraction tiles
NT = T // 128           # 16 row tiles
QCHUNK = 512
NQC = T // QCHUNK       # 4 q-chunks
JPER = QCHUNK // 128    # 4 j-tiles per q-chunk
SCALE = float(1.0 / np.sqrt(D))
N_WARMUP = 34           # 128-col PE p-state ramp matmuls during initial DMA

FP = mybir.dt.float32
BF = mybir.dt.bfloat16


def build_nc():
    nc = bacc.Bacc("TRN2", target_bir_lowering=False)
    xT_h = nc.dram_tensor("xT", [128, CT, T], BF, kind="ExternalInput")
    wqk_h = nc.dram_tensor("wqk", [128, CT, 128], BF, kind="ExternalInput")
    wv_h = nc.dram_tensor("wv", [128, CT, D], BF, kind="ExternalInput")
    y_h = nc.dram_tensor("y", [128, NT, D], FP, kind="ExternalOutput")

    with tile.TileContext(nc) as tc:
        with (
            tc.tile_pool(name="const", bufs=1) as const,
            tc.tile_pool(name="pt", bufs=6) as ptp,      # [128,2,512] bf16
            tc.tile_pool(name="dve", bufs=2) as dvp,
            tc.tile_pool(name="ps_s2", bufs=2, space="PSUM") as ps_s2,  # 2x2 banks
            tc.tile_pool(name="ps_pq", bufs=2, space="PSUM") as ps_pq,  # 2 banks
            tc.tile_pool(name="ps_pv", bufs=1, space="PSUM") as ps_pv,  # 1 bank
            tc.tile_pool(name="ps_o", bufs=1, space="PSUM") as ps_o,    # 1 bank
        ):
            # ---- constants (wu first: it gates the PE warmup) ----
            wu = const.tile([128, 128], BF, tag="wu")
            nc.gpsimd.memset(wu, 0.0)
            tri = const.tile([128, 128], BF, tag="tri")  # tri[p,f]=1.0 iff f>=p
            make_upper_triangular(nc, tri, val=1.0, diag=True)
            # preload the Exp activation table while DMAs are in flight
            dum = const.tile([1, 2], BF, tag="dum")
            nc.scalar.activation(
                dum[0:1, 0:1], wu[0:1, 0:1], mybir.ActivationFunctionType.Exp
            )

            wqk_sb = const.tile([128, CT, 128], BF, tag="wqk")
            wv_sb = const.tile([128, CT, D], BF, tag="wv")
            xT_sb = const.tile([128, CT, T], BF, tag="xT")

            # input DMAs, all upfront on SP, in consumption order; ct-quarters
            # let the qk contraction start on the first arriving piece
            def xdma(cu):
                sl = slice(cu * QCHUNK, (cu + 1) * QCHUNK)
                for q in range(4):
                    nc.sync.dma_start(
                        out=xT_sb[:, 2 * q : 2 * q + 2, sl],
                        in_=xT_h[:, 2 * q : 2 * q + 2, sl],
                    )

            nc.sync.dma_start(out=wqk_sb, in_=wqk_h[:, :, :])
            xdma(0)
            nc.sync.dma_start(out=wv_sb, in_=wv_h[:, :, :])
            for cu in range(1, NQC):
                xdma(cu)

            # q/k live on partitions 64:128 (psum high half evacuates with no
            # partition shift); k's low half goes through a staging tile and an
            # SBUF->SBUF DMA (the only engine-free way to shift partitions)
            qT = const.tile([128, T], BF, tag="qT")
            kT = const.tile([128, T], BF, tag="kT")
            kst = const.tile([64, T], BF, tag="kst")
            V = const.tile([128, NT, D + 1], BF, tag="V")  # col D = ones
            nc.gpsimd.memset(V[:, :, D], 1.0)
            out_sb = const.tile([128, NT, D], FP, tag="out")

            # ---- PE p-state warmup while the first DMA is in flight ----
            ps_junk = ps_s2.tile([128, 2, QCHUNK], FP, tag="s2")
            for w in range(N_WARMUP):
                nc.tensor.matmul(
                    ps_junk[:, 0, 0:128], wu, wu, start=True, stop=True
                )

            proj_psum = {}

            proj_psums = {}

            def proj_qk(cu, lo_ct, hi_ct):
                """qk (W-stationary), emitted in ct-halves so the PE order can
                match the x-quarter DMA arrivals; on the last half, q/k are
                evacuated to bf16: one full-width DVE copy (q rows 64: feed S
                directly), k rows :64 partition-shifted into kT by an
                SBUF->SBUF DMA on the idle Pool queue."""
                sl = slice(cu * QCHUNK, (cu + 1) * QCHUNK)
                if lo_ct == 0:
                    proj_psums[cu] = ps_pq.tile(
                        [128, QCHUNK], FP, tag="pq", name=f"p_qk{cu}"
                    )
                p_qk = proj_psums[cu]
                for ct in range(lo_ct, hi_ct):
                    nc.tensor.matmul(
                        p_qk,
                        wqk_sb[:, ct, :],
                        xT_sb[:, ct, sl],
                        start=(ct == 0),
                        stop=(ct == CT - 1),
                    )
                if hi_ct < CT:
                    return
                nc.vector.tensor_copy(qT[:, sl], p_qk)
                if cu < NQC - 1:
                    # ACT has boundary idle here, and the direct shifted copy
                    # is the lowest-latency path to kT
                    nc.scalar.copy(kT[64:128, sl], p_qk[0:64, :])
                else:
                    # last chunk: ACT is saturated, but the DMA engines are
                    # free (all x transfers done) - shift via SBUF->SBUF DMA
                    nc.gpsimd.dma_start(out=kT[64:128, sl], in_=qT[0:64, sl])

            def proj_v(cu):
                """v (x-stationary, natural layout) + evacuation into V."""
                p_v = ps_pv.tile([128, JPER, D], FP, tag="pv")
                for tt in range(JPER):
                    tsl = slice(cu * QCHUNK + tt * 128, cu * QCHUNK + (tt + 1) * 128)
                    for ct in range(CT):
                        nc.tensor.matmul(
                            p_v[:, tt, :],
                            xT_sb[:, ct, tsl],
                            wv_sb[:, ct, :],
                            start=(tt == 0 and ct == 0),
                            stop=(tt == JPER - 1 and ct == CT - 1),
                            skip_group_check=True,
                        )
                nc.vector.tensor_copy(V[:, cu * JPER : (cu + 1) * JPER, 0:D], p_v)

            # ---- global attention unit stream: all units are j-tile PAIRS --
            # unit key (cu, j0): j-tiles (j0, j0+1) against q-chunk cu
            units = {}
            for cu in range(NQC):
                for j in range(0, (cu + 1) * JPER, 2):
                    units[(cu, j)] = (cu, (j, j + 1))
            # last two diagonal tiles as singles: shortens the drain chain
            units[(3, 14)] = (3, (14,))
            units[(3, 15)] = (3, (15,))

            p_outs = {}

            def s_unit(key):
                cu, u = units[key]
                # both tiles of the pair are computed from the pair's lowest
                # causal column so a single exp AP covers them; columns left
                # of a tile's own boundary are never read by AV
                lo = max(u[0] - cu * JPER, 0) * 128
                p_s = ps_s2.tile([128, 2, QCHUNK], FP, tag="s2")
                pt = ptp.tile([128, 2, QCHUNK], BF, tag="pt")
                for z, j in enumerate(u):
                    nc.tensor.matmul(
                        p_s[:, z, lo:QCHUNK],
                        kT[64:128, j * 128 : (j + 1) * 128],
                        qT[64:128, cu * QCHUNK + lo : (cu + 1) * QCHUNK],
                        start=True,
                        stop=True,
                    )
                nz = len(u)
                nc.scalar.activation(
                    pt[:, 0:nz, lo:QCHUNK],
                    p_s[:, 0:nz, lo:QCHUNK],
                    mybir.ActivationFunctionType.Exp,
                    scale=SCALE,
                )
                for z, j in enumerate(u):
                    i_d = j - cu * JPER
                    if i_d >= 0:
                        nc.vector.tensor_mul(
                            pt[:, z, i_d * 128 : (i_d + 1) * 128],
                            pt[:, z, i_d * 128 : (i_d + 1) * 128],
                            tri,
                        )
                return pt

            def av_unit(key, pt):
                cu, u = units[key]
                last = cu == NQC - 1
                n_jt = cu * JPER + JPER
                if key[1] == 0:
                    p_outs[cu] = ps_o.tile([128, JPER, D + 1], FP, tag="o", name=f"p_out{cu}")
                    if last:
                        # the pv bank is free after V3's evacuation: give the
                        # last chunk's tiles 2-3 their own bank so the early
                        # finalize of tiles 0-1 cannot WAR-block the final AVs
                        p_outs["3b"] = ps_pv.tile(
                            [128, 2, D + 1], FP, tag="pv", name="p_out3b"
                        )
                for z, j in enumerate(u):
                    pj = pt[:, z, :]
                    i_d = j - cu * JPER
                    for qi in range(max(i_d, 0), JPER):
                        if last and qi >= 2:
                            tgt = p_outs["3b"][:, qi - 2, :]
                            st = j == 0 and qi == 2
                            sp = j == n_jt - 1 and qi == JPER - 1
                        else:
                            tgt = p_outs[cu][:, qi, :]
                            st = j == 0 and qi == 0
                            sp = (j == n_jt - 1 and qi == JPER - 1) or (
                                last and j == 13 and qi == 1
                            )
                        nc.tensor.matmul(
                            tgt,
                            pj[:, qi * 128 : (qi + 1) * 128],
                            V[:, j, :],
                            start=st,
                            stop=sp,
                            skip_group_check=True,
                        )

            def finalize(cu, lo, hi, dma_lo=0):
                if cu == NQC - 1 and lo >= 2:
                    p_out, off = p_outs["3b"], 2
                else:
                    p_out, off = p_outs[cu], 0
                rec = dvp.tile([128, hi - lo], FP, tag=f"rec{hi - lo}")
                nc.vector.reciprocal(rec, p_out[:, lo - off : hi - off, D])
                for qi in range(lo, hi):
                    nc.vector.tensor_scalar_mul(
                        out_sb[:, cu * JPER + qi, :],
                        p_out[:, qi - off, 0:D],
                        rec[:, qi - lo : qi - lo + 1],
                    )
                if dma_lo is None:
                    return
                nc.sync.dma_start(
                    out=y_h[:, cu * JPER + dma_lo : cu * JPER + hi, :],
                    in_=out_sb[:, cu * JPER + dma_lo : cu * JPER + hi, :],
                )

            # ---- explicit hand-scheduled action stream -------------------
            # Engines execute in emission order, so cross-chunk interleaving
            # is encoded directly: diagonal units (which wait on the staged
            # kT DMA) are deferred behind later chunks' off-diagonal units;
            # qk halves are placed where their x quarters have landed.
            S, AV = "s", "av"
            actions = [
                ("qk", 0, 0, 8), ("v", 0),
                ("qk", 1, 0, 2), (S, 0, 0), ("qk", 1, 2, 4), (S, 0, 2),
                ("qk", 1, 4, 6), ("qk", 1, 6, 8),
                (AV, 0, 0), (AV, 0, 2), ("fin", 0, 0, 4),
                ("qk", 2, 0, 2), ("v", 1),
                (S, 1, 0), (S, 1, 2),
                ("qk", 2, 2, 4), ("qk", 2, 4, 6),
                (AV, 1, 0), (AV, 1, 2),
                ("qk", 2, 6, 8),
                (S, 2, 0),
                ("qk", 3, 0, 2), ("v", 2),
                (S, 1, 4), (S, 2, 2),
                ("qk", 3, 2, 4),
                (AV, 1, 4),
                ("qk", 3, 4, 6),
                (S, 1, 6), (S, 2, 4),
                ("qk", 3, 6, 8),
                (AV, 1, 6), ("fin", 1, 0, 4),
                (S, 3, 0), (S, 2, 6),
                (AV, 2, 0), (AV, 2, 2), (AV, 2, 4),
                (S, 3, 2), (S, 2, 8),
                ("v", 3),
                (AV, 2, 6),
                (S, 3, 4), (S, 2, 10),
                (AV, 2, 8), (AV, 2, 10), ("fin", 2, 0, 4),
                (S, 3, 6), (S, 3, 8),
                (AV, 3, 0), (AV, 3, 2),
                (S, 3, 10), (S, 3, 12),
                (AV, 3, 4), (AV, 3, 6), (AV, 3, 8),
                (S, 3, 14), (S, 3, 15),
                (AV, 3, 10), (AV, 3, 12), ("fin", 3, 0, 2),
                (AV, 3, 14), ("fin", 3, 2, 3, None),
                (AV, 3, 15), ("fin", 3, 3, 4, 2),
            ]
            pts = {}
            for act in actions:
                if act[0] == "qk":
                    proj_qk(act[1], act[2], act[3])
                elif act[0] == "v":
                    proj_v(act[1])
                elif act[0] == S:
                    pts[act[1:]] = s_unit(act[1:])
                elif act[0] == AV:
                    av_unit(act[1:], pts.pop(act[1:]))
                else:
                    finalize(act[1], act[2], act[3], *act[4:])
            assert not pts

    nc.finalize()
    return nc


_NC_CACHE = None
LAST_RESULTS = None


def _pack(w, cols):
    # [C, cols] -> [128, CT, cols] with partition p holding rows {ct*128+p}
    return np.ascontiguousarray(
        np.asarray(w, np.float32).reshape(CT, 128, cols).transpose(1, 0, 2)
    ).astype(ml_dtypes.bfloat16)


def kernel(x, Wq, Wk, Wv, trace=False, **run_kwargs):
    global _NC_CACHE, LAST_RESULTS
    x = np.asarray(x, dtype=np.float32)
    # k in the psum low half, q in the high half (see build_nc)
    wqk = _pack(np.concatenate([np.asarray(Wk, np.float32),
                                np.asarray(Wq, np.float32)], axis=1), 128)
    wv = _pack(Wv, D)

    if _NC_CACHE is None:
        _NC_CACHE = build_nc()
    nc = _NC_CACHE

    in_maps = []
    for b in range(N_CORES):
        # xT[p, ct, t] = x[b, t, ct*128+p]
        xT = np.ascontiguousarray(
            x[b].T.reshape(CT, 128, T).transpose(1, 0, 2)
        ).astype(ml_dtypes.bfloat16)
        in_maps.append({"xT": xT, "wqk": wqk, "wv": wv})

    res = run_bass_kernel_spmd(
        nc, in_maps, core_ids=list(range(N_CORES)), trace=trace, **run_kwargs
    )
    LAST_RESULTS = res
    out = np.empty((N_CORES, T, D), dtype=np.float32)
    for b in range(N_CORES):
        y = res.results[b]["y"]  # [128, NT, D]
        out[b] = np.asarray(y, dtype=np.float32).transpose(1, 0, 2).reshape(T, D)
    return out


if __name__ == "__main__":
    rng = np.random.default_rng(0)
    x = rng.standard_normal((B, T, C), dtype=np.float32)
    s = 1.0 / np.sqrt(C)
    Wq = rng.standard_normal((C, D), dtype=np.float32) * s
    Wk = rng.standard_normal((C, D), dtype=np.float32) * s
    Wv = rng.standard_normal((C, D), dtype=np.float32) * s
    out = kernel(x, Wq, Wk, Wv)
    print("out", out.shape, out.dtype, float(np.abs(out).max()))
